# revision 8
# baseline (speedup 1.0000x reference)
"""LIF bank (nn_LIFBank_17059610100011) Trainium2 Bass kernel.

Per-lane recurrence (T sequential steps), data-parallel over B*N lanes:
8 cores x 4096 lanes ([128 partitions, 32 free] tiles).

v4: software-pipelined 6-op step. Refractory gating is rewritten as
u_eff_t = u_t * (1 - s_{t-1}) * (1 - s_{t-2})   (exact: ref>0 <=> spike in
last 2 steps), split into two fused ops so every DVE instruction's inputs
are produced >=2 instructions earlier (hides the ~60ns SBUF write->read
turnaround). Step window order:

    W_t   = alpha*V_{t-1} + M_t          (scalar_tensor_tensor)
    P_t+1 = u_{t+1} * (1 - S_{t-1})      (custom LIF_MUL_COMPL)
    S_t   = (W_t >= theta_{t-1})         (tensor_tensor is_ge) -> spikes out
    V_t   = W_t - S_t*theta_{t-1}        (custom LIF_SOFTRESET) -> v_hist out
    M_t+1 = P_{t+1} * (1 - S_t)          (custom LIF_MUL_COMPL)
    T_t   = (theta*BETA + c) + GAMMA*S_t (custom LIF_THETASPIKE)

fp32 rounding order matches the jax reference exactly (mult-then-add,
two roundings; c = tb*(1-BETA) precomputed on host).
"""

import numpy as np

ALPHA = 0.95
BETA = 0.995   # THETA_DECAY
GAMMA = 0.35   # THETA_INC

B, N, T = 16, 2048, 1000
NCORES = 8
NSH = N // NCORES          # 256 neurons per core
P, F = 128, 32             # lanes per core = P*F = B*NSH = 4096
TC = 125                   # timesteps per DMA chunk

_CACHE = {}


def _register_custom_ops():
    import concourse.dve_ops as dvo
    from concourse.dve_spec import (
        Spec, Src0, Src1, C0, C1, C2, One, select, lower, _has_src1,
    )
    from concourse.dve_uop import DveOpSpec

    if "LIF_MUL_COMPL" in dvo._SUB_OPCODE_FOR_NAME:
        return {o.name: o for o in dvo.OPS if o.name.startswith("LIF_")}

    specs = {
        "LIF_MUL_COMPL": Spec(
            body=Src0 * (One - Src1),
            reference=lambda in0, in1, s0, s1, imm2: (in0 * (1.0 - in1)).astype(np.float32),
        ),
        "LIF_SOFTRESET": Spec(
            body=select(Src0 < Src1, Src0, Src0 - Src1),
            reference=lambda in0, in1, s0, s1, imm2: np.where(in0 < in1, in0, in0 - in1).astype(np.float32),
        ),
        "LIF_THETASPIKE": Spec(
            body=(Src0 * C0 + C1) + (Src1 >= Src0) * C2,
            reference=lambda in0, in1, s0, s1, imm2: (
                (in0 * np.float32(s0) + np.float32(s1))
                + (in1 >= in0).astype(np.float32) * np.float32(imm2)
            ).astype(np.float32),
        ),
    }
    new_ops = []
    base = len(dvo.OPS)
    for i, (name, spec) in enumerate(specs.items()):
        opcode = dvo._CUSTOM_DVE_ROW_BASE + base + i
        shas = {}
        for ver in ("v3", "v4"):
            uops = lower(spec, ver=ver)
            shas[ver] = DveOpSpec(
                name=name, opcode=opcode, uops=uops, rd1_en=_has_src1(spec)
            ).sha(ver)
        dvo._SUB_OPCODE_FOR_NAME[name] = opcode
        new_ops.append(dvo.DveOp(name, spec, subdim=False, uops_sha=shas))
    dvo.OPS.extend(new_ops)
    dvo.CUSTOM_DVE_SPECS.update({o.name: o.spec for o in new_ops})
    return {o.name: o for o in new_ops}


def _build_nc(t_total, tc, c_imm):
    import concourse.bacc as bacc
    import concourse.mybir as mybir
    import concourse.tile as tile

    ops = _register_custom_ops()
    MC, SR, TS = ops["LIF_MUL_COMPL"], ops["LIF_SOFTRESET"], ops["LIF_THETASPIKE"]

    f32 = mybir.dt.float32
    op = mybir.AluOpType

    nc = bacc.Bacc("TRN2", target_bir_lowering=False, num_devices=NCORES)
    u_d = nc.dram_tensor("u", [P, F, t_total], f32, kind="ExternalInput")
    tb_d = nc.dram_tensor("tb", [P, F], f32, kind="ExternalInput")
    s_d = nc.dram_tensor("s", [P, F, t_total], f32, kind="ExternalOutput")
    v_d = nc.dram_tensor("v", [P, F, t_total], f32, kind="ExternalOutput")

    nchunks = t_total // tc
    assert nchunks * tc == t_total
    vec = nc.vector

    with tile.TileContext(nc) as tc_ctx:
        with (
            tc_ctx.tile_pool(name="state", bufs=1) as st,
            tc_ctx.tile_pool(name="ustage", bufs=3) as upool,
            tc_ctx.tile_pool(name="sstage", bufs=3) as sbpool,
            tc_ctx.tile_pool(name="vstage", bufs=3) as vbpool,
        ):
            zero = st.tile([P, F], f32, tag="zero", name="zero")
            th = [st.tile([P, F], f32, tag=f"th{i}", name=f"th{i}") for i in range(4)]
            wr = [st.tile([P, F], f32, tag=f"w{i}", name=f"w{i}") for i in range(2)]
            pr = [st.tile([P, F], f32, tag=f"p{i}", name=f"p{i}") for i in range(2)]
            mr = [st.tile([P, F], f32, tag=f"m{i}", name=f"m{i}") for i in range(2)]

            vec.memset(zero[:], 0.0)
            nc.sync.dma_start(th[3][:], tb_d[:, :])  # theta_{-1} = tb

            ub, sb, vb = {}, {}, {}

            def load_chunk(c):
                if c < nchunks and c not in ub:
                    ub[c] = upool.tile([P, F, tc], f32, tag="ub", name=f"ub{c}")
                    nc.sync.dma_start(ub[c][:], u_d[:, :, c * tc:(c + 1) * tc])

            def u_at(t):
                return ub[t // tc][:, :, t % tc]

            def s_at(t):
                return zero[:, :] if t < 0 else sb[t // tc][:, :, t % tc]

            def v_at(t):
                return zero[:, :] if t < 0 else vb[t // tc][:, :, t % tc]

            load_chunk(0)

            # prologue: P_0 = u_0*(1-0), M_0 = P_0*(1-0)
            vec._custom_dve(MC, out=pr[0][:], in0=u_at(0), in1=zero[:, :])
            vec._custom_dve(MC, out=mr[0][:], in0=pr[0][:], in1=zero[:, :])

            for t in range(t_total):
                c = t // tc
                if t % tc == 0:
                    sb[c] = sbpool.tile([P, F, tc], f32, tag="sb", name=f"sbc{c}")
                    vb[c] = vbpool.tile([P, F, tc], f32, tag="vb", name=f"vbc{c}")
                    load_chunk(c + 1)

                thp = th[(t - 1) % 4][:, :]   # theta_{t-1}
                w = wr[t % 2][:]

                # W_t = alpha*V_{t-1} + M_t
                vec.scalar_tensor_tensor(
                    out=w, in0=v_at(t - 1), scalar=ALPHA, in1=mr[t % 2][:],
                    op0=op.mult, op1=op.add,
                )
                # P_{t+1} = u_{t+1} * (1 - S_{t-1})
                if t + 1 < t_total:
                    vec._custom_dve(
                        MC, out=pr[(t + 1) % 2][:], in0=u_at(t + 1), in1=s_at(t - 1),
                    )
                # S_t = (W_t >= theta_{t-1})
                vec.tensor_tensor(out=sb[c][:, :, t % tc], in0=w, in1=thp, op=op.is_ge)
                # V_t = soft reset
                vec._custom_dve(SR, out=vb[c][:, :, t % tc], in0=w, in1=thp)
                # M_{t+1} = P_{t+1} * (1 - S_t)
                if t + 1 < t_total:
                    vec._custom_dve(
                        MC, out=mr[(t + 1) % 2][:], in0=pr[(t + 1) % 2][:],
                        in1=sb[c][:, :, t % tc],
                    )
                # theta_t = (theta_{t-1}*BETA + c) + GAMMA*S_t
                vec._custom_dve(
                    TS, out=th[t % 4][:], in0=thp, in1=w,
                    s0=BETA, s1=c_imm, imm2=GAMMA,
                )

                if t % tc == tc - 1:
                    nc.sync.dma_start(s_d[:, :, c * tc:(c + 1) * tc], sb[c][:])
                    nc.sync.dma_start(v_d[:, :, c * tc:(c + 1) * tc], vb[c][:])

    nc.compile()
    return nc


def _get_nc(t_total, tc, c_imm):
    key = (t_total, tc, float(c_imm))
    if key not in _CACHE:
        _CACHE[key] = _build_nc(t_total, tc, c_imm)
    return _CACHE[key]


def _shard_inputs(u, theta_base, t_total):
    u = np.asarray(u, dtype=np.float32)
    tb = np.asarray(theta_base, dtype=np.float32)[0, :, 0]  # [N]
    in_maps = []
    for c in range(NCORES):
        lo, hi = c * NSH, (c + 1) * NSH
        uc = np.ascontiguousarray(
            u[:, lo:hi, :t_total].reshape(B, NSH // F, F, t_total).reshape(P, F, t_total)
        )
        tbc = np.tile(tb[lo:hi].reshape(NSH // F, F), (B, 1)).astype(np.float32)
        in_maps.append({"u": uc, "tb": tbc})
    return in_maps


def _unshard(res, t_total):
    s_full = np.empty((B, N, t_total), dtype=np.float32)
    v_full = np.empty((B, N, t_total), dtype=np.float32)
    for c in range(NCORES):
        lo, hi = c * NSH, (c + 1) * NSH
        s_full[:, lo:hi, :] = res[c]["s"].reshape(B, NSH // F, F, t_total).reshape(B, NSH, t_total)
        v_full[:, lo:hi, :] = res[c]["v"].reshape(B, NSH // F, F, t_total).reshape(B, NSH, t_total)
    return s_full, v_full


def _host_fallback(u, theta_base):
    """Exact numpy step simulation; only used if theta_base is non-uniform."""
    u = np.asarray(u, np.float32)
    b, n, t = u.shape
    tb = np.asarray(theta_base, np.float32)[0, :, 0]
    v = np.zeros((b, n), np.float32)
    theta = np.broadcast_to(tb, (b, n)).astype(np.float32).copy()
    ref = np.zeros((b, n), np.float32)
    c = (tb * np.float32(1.0 - BETA)).astype(np.float32)
    ss = np.empty((b, n, t), np.float32)
    vs = np.empty((b, n, t), np.float32)
    for i in range(t):
        u_eff = np.where(ref > 0, np.float32(0.0), u[:, :, i])
        v = (np.float32(ALPHA) * v + u_eff).astype(np.float32)
        s = (v >= theta).astype(np.float32)
        v = (v - s * theta).astype(np.float32)
        ref = np.where(s > 0, np.float32(2.0), np.maximum(ref - 1.0, 0.0).astype(np.float32))
        theta = ((theta * np.float32(BETA) + c) + np.float32(GAMMA) * s).astype(np.float32)
        ss[:, :, i] = s
        vs[:, :, i] = v
    return ss, vs


def run(u, theta_base, t_total=T, tc=TC, trace=False):
    from concourse.bass_utils import run_bass_kernel_spmd

    tb = np.asarray(theta_base, dtype=np.float32)
    c_imm = float(np.float32(tb.flat[0]) * np.float32(1.0 - BETA))

    nc = _get_nc(t_total, tc, c_imm)
    in_maps = _shard_inputs(u, theta_base, t_total)
    res = run_bass_kernel_spmd(nc, in_maps, core_ids=list(range(NCORES)), trace=trace)
    s_full, v_full = _unshard(res.results, t_total)
    return (s_full, v_full), res


def kernel(u, theta_base):
    tb = np.asarray(theta_base, dtype=np.float32)
    if not np.all(tb == tb.flat[0]):
        return _host_fallback(u, theta_base)
    (s_full, v_full), _ = run(u, theta_base)
    return s_full, v_full
